# revision 16
# baseline (speedup 1.0000x reference)
"""Grouped SwiGLU expert MLP (MoE) on 8 Trainium2 NeuronCores.

Problem: sorted_x [32768, 512] f32, tokens pre-sorted by expert into 8 equal
contiguous segments of 4096 tokens; per-expert SwiGLU MLP
    h12 = x_e @ w12[e].T          (4096, 2816)
    h   = silu(h12[:, :1408]) * h12[:, 1408:]
    out = h @ w3[e].T             (4096, 512)

Sharding: pure expert parallelism — core e owns expert e's weights and its
4096-token segment (sliced host-side from expert_starts), so no device-side
collectives are needed; the host concatenates the per-core outputs.

Device layout is feature-major throughout ("contraction dim on partitions"),
which makes both GEMMs transpose-free on chip:
    xt   = x_e.T   [512, 4096]  fp16
    w12p = w12.T   pre-tiled    fp16  (see below)
    w3t  = w3.T    [1408, 512]  fp16
    outT = out.T   [512, 4096]  fp16  (host transposes + upcasts back)
GEMM1 produces H12^T tiles [128h, Nt] (PSUM f32), SwiGLU runs on ACT+DVE into
fp16 H^T tiles, GEMM2 consumes them directly. fp16 operands run the PE at
1 cycle/row; accumulation is always f32 in PSUM.

The PE stream (1056 matmuls x 512 moving rows = 225.3us warm) is the hard
floor; everything else is edge management:
  * warm-up matmuls on a memset scratch tile run during the fixed ~6.6us
    NEFF prologue + first-DMA window, so HAM un-throttles (1.2->2.4 GHz)
    before the real stream begins and the PE never sits idle at the head;
  * w12 is repacked host-side into PE consumption order — 22 stationary
    tile-groups t=(g0,u0,g1,u1,...) of [kd][128 cols], 128KB each with 1KB
    contiguous runs per partition (column-sliced chunks of the plain
    transposed layout have 256B runs and run at ~4x lower DMA bandwidth,
    which starved the PE for ~10us at the head in earlier revisions);
  * input DMAs are split/ordered by consumption time across the two queues
    (sync=SP: w12 tile stream + output stores; gpsimd=SWDGE: x0 per-kd,
    then w3, then x1..7);
  * GEMM2 runs do-major after each block's hh loop (not LAG-interleaved),
    so acc[do] banks finish one at a time and the PSUM->SBUF copy + output
    DMA of each do overlaps the remaining matmul stream; copies split in
    half across ACT and DVE; the very last store is split across both DMA
    queues so only ~64KB trails the final matmul;
  * outputs store as fp16 (adds ~1e-4 rel err vs the 2e-2 budget).
"""

import os

import numpy as np
import ml_dtypes

import concourse.bass as bass
import concourse.mybir as mybir
import concourse.tile as tile
from concourse import bacc
from concourse.bass_utils import run_bass_kernel_spmd

N_CORES = 8
D = 512  # d_model
H = 1408  # hidden
TWOH = 2 * H
TPE = 4096  # tokens per expert
NT = 512  # token block (matmul moving free dim, one PSUM bank in f32)
KD = D // 128  # 4 contraction tiles over d
KH = H // 128  # 11 contraction tiles over h
NB = TPE // NT  # token blocks
NTILE = 2 * KH  # 22 stationary tile-groups of w12 in consumption order

F16 = mybir.dt.float16
F32 = mybir.dt.float32
NP_F16 = np.dtype(np.float16)

N_WARMUP = 48  # LDW+MM pairs on scratch data before the real stream

# Results of a traced run (test harness reads these).
last_exec_time_ns = None
last_trace_path = None


def _build():
    # Bacc (not plain Bass): its compile() pass pipeline legalizes sync
    # waits (>=2 waits per instruction are split into event-sem chains),
    # which this image's walrus requires.
    nc = bacc.Bacc("TRN2", target_bir_lowering=False, debug=False, num_devices=N_CORES)
    xt = nc.dram_tensor("xt", [D, TPE], F16, kind="ExternalInput")
    w12p = nc.dram_tensor("w12p", [128, NTILE * KD * 128], F16, kind="ExternalInput")
    w3t = nc.dram_tensor("w3t", [H, D], F16, kind="ExternalInput")
    outT = nc.dram_tensor("outT", [D, TPE], F16, kind="ExternalOutput")

    with tile.TileContext(nc) as tc:
        with (
            tc.tile_pool(name="weights", bufs=1) as wpool,
            tc.tile_pool(name="xin", bufs=1) as xpool,
            tc.tile_pool(name="ht", bufs=2) as hpool,
            tc.tile_pool(name="swi", bufs=4) as spool,
            tc.tile_pool(name="ot", bufs=6) as opool,
            tc.tile_pool(name="pg", bufs=2, space=bass.MemorySpace.PSUM) as pgate,
            tc.tile_pool(name="pu", bufs=2, space=bass.MemorySpace.PSUM) as pup,
            tc.tile_pool(name="po", bufs=1, space=bass.MemorySpace.PSUM) as pacc,
        ):
            # w12s[p, t, kd, c]: t = 2*hh (gate) / 2*hh+1 (up)
            w12s = wpool.tile([128, NTILE, KD, 128], F16)
            w3s = wpool.tile([128, KH, D], F16)
            xs = xpool.tile([128, KD, TPE], F16)
            warm = wpool.tile([128, 128], F16)

            xt_r = xt[:, :].rearrange("(kd p) t -> p kd t", p=128)
            w12_r = w12p[:, :].rearrange("p (t kd c) -> p t kd c", kd=KD, c=128)
            w3_r = w3t[:, :].rearrange("(kh p) d -> p kh d", p=128)
            outT_r = outT[:, :].rearrange("(do p) t -> p do t", p=128)

            # Warm-up scratch init must be the FIRST gpsimd instruction —
            # everything later on that sequencer queues behind ~10us of DMA
            # issue slots, and the PE warm-up matmuls depend on it.
            nc.gpsimd.memset(warm[:], 0)

            # Each DMA_DIRECT2D costs ~650ns of sequencer issue time and the
            # two dynamic queues (sync=SP, gpsimd=Pool/SWDGE) split ~358GB/s
            # of HBM read bandwidth, so chunks are sized/ordered to land just
            # ahead of PE consumption: w12 tile t is consumed at roughly
            # first_mm + 0.87*t us, w3 at +19us, x block tb at +28.2*tb us.
            # x0 as ONE 512KB transfer on gpsimd, alone until it completes
            # (early aggregate DMA bw is only ~250-300GB/s while both queues
            # ramp; anything sharing the queue delays the first real matmul).
            # w12 tiles split across BOTH queues in consumption order; w3 and
            # x1..7 queue behind (w3 isn't needed until block 0's GEMM2).
            def dma_w12(q, t0, t1):
                q.dma_start(out=w12s[:, t0:t1, :, :], in_=w12_r[:, t0:t1, :, :])

            nc.gpsimd.dma_start(out=xs[:, :, 0:NT], in_=xt_r[:, :, 0:NT])
            for t0, t1 in [(0, 1), (1, 2), (2, 4), (6, 10), (14, 18)]:
                dma_w12(nc.sync, t0, t1)
            for t0, t1 in [(4, 6), (10, 14), (18, NTILE)]:
                dma_w12(nc.gpsimd, t0, t1)
            nc.gpsimd.dma_start(out=w3s[:, 0:6, :], in_=w3_r[:, 0:6, :])
            nc.gpsimd.dma_start(out=w3s[:, 6:KH, :], in_=w3_r[:, 6:KH, :])
            for tb in range(1, NB):
                nc.gpsimd.dma_start(
                    out=xs[:, :, tb * NT : (tb + 1) * NT],
                    in_=xt_r[:, :, tb * NT : (tb + 1) * NT],
                )

            # Warm-up: PE activity with no DMA dependency, issued first so it
            # runs during the prologue/first-chunk window and flips HAM to
            # 8/8 before the real matmuls.  Garbage values, never read;
            # shares the ps_g tag so it cycles inside pgate's 2 banks.
            wps = pgate.tile([128, NT], F32, name="ps_g", tag="ps_g")
            for _ in range(N_WARMUP):
                nc.tensor.matmul(wps[:, 0:128], warm[:], warm[:], start=True, stop=True)

            for tb in range(NB):
                tsl = bass.ts(tb, NT)
                ht = hpool.tile([128, KH, NT], F16)
                acc = [
                    pacc.tile([128, NT], F32, name=f"acc{do}", tag=f"acc{do}")
                    for do in range(KD)
                ]

                # GEMM1 + SwiGLU, hh-major.  Gate/up matmuls interleaved per
                # kd so the head of block 0 tracks the per-kd x0 DMA arrivals.
                for hh in range(KH):
                    ps_g = pgate.tile([128, NT], F32)
                    ps_u = pup.tile([128, NT], F32)
                    for kd in range(KD):
                        nc.tensor.matmul(
                            ps_g[:],
                            w12s[:, 2 * hh, kd, :],
                            xs[:, kd, tsl],
                            start=(kd == 0),
                            stop=(kd == KD - 1),
                        )
                        nc.tensor.matmul(
                            ps_u[:],
                            w12s[:, 2 * hh + 1, kd, :],
                            xs[:, kd, tsl],
                            start=(kd == 0),
                            stop=(kd == KD - 1),
                        )
                    sil = spool.tile([128, NT], F32)
                    nc.scalar.activation(
                        sil[:], ps_g[:], mybir.ActivationFunctionType.Silu
                    )
                    nc.vector.tensor_mul(ht[:, hh, :], sil[:], ps_u[:])

                # GEMM2 do-major: each acc bank finishes 11 matmuls before the
                # next starts, so its copy+store overlap the remaining stream.
                # The kh chain ends at kh=10 whose ht lands ~1.4us after the
                # last GEMM1 matmul — covered by the 10 preceding matmuls.
                for do in range(KD):
                    last = tb == NB - 1 and do == KD - 1
                    ot = opool.tile([128, NT], F16)
                    hn = NT // 2
                    t0 = tb * NT
                    if not last:
                        for kh in range(KH):
                            nc.tensor.matmul(
                                acc[do][:],
                                w3s[:, kh, do * 128 : (do + 1) * 128],
                                ht[:, kh, :],
                                start=(kh == 0),
                                stop=(kh == KH - 1),
                            )
                        # PSUM copies on one engine each (bass serializes
                        # same-bank ACT+DVE access), alternating engines
                        if do % 2 == 0:
                            nc.scalar.copy(ot[:], acc[do][:])
                        else:
                            nc.vector.tensor_copy(ot[:], acc[do][:])
                        nc.sync.dma_start(out=outT_r[:, do, tsl], in_=ot[:])
                    else:
                        # the very last chain runs as shrinking sub-chains
                        # (256, 128, 128 tokens) so each piece's copy+store
                        # overlap the next piece's matmuls and only ~16KB
                        # trails the final MM.  Later pieces accumulate in
                        # acc[0]/acc[1]'s banks (copied out long ago) — PE
                        # may not write a bank a copy is still reading, and
                        # same-bank ACT/DVE copies serialize, so each piece
                        # gets its own bank + alternating copy engine.
                        # All stores go via the sync queue — it has been
                        # streaming stores all along; the gpsimd queue has
                        # been idle for ~200us and costs ~1us to re-ramp.
                        qn = NT // 4
                        for (lo, hi), pt, eng in (
                            ((0, hn), acc[do], nc.scalar),
                            ((hn, hn + qn), acc[0], nc.vector),
                            ((hn + qn, NT), acc[1], nc.scalar),
                        ):
                            w = hi - lo
                            for kh in range(KH):
                                nc.tensor.matmul(
                                    pt[:, 0:w],
                                    w3s[:, kh, do * 128 : (do + 1) * 128],
                                    ht[:, kh, lo:hi],
                                    start=(kh == 0),
                                    stop=(kh == KH - 1),
                                )
                            if eng is nc.scalar:
                                eng.copy(ot[:, lo:hi], pt[:, 0:w])
                            else:
                                eng.tensor_copy(ot[:, lo:hi], pt[:, 0:w])
                            nc.sync.dma_start(
                                out=outT_r[:, do, t0 + lo : t0 + hi],
                                in_=ot[:, lo:hi],
                            )
    nc.compile()
    return nc


_nc_cache = None


def _get_nc():
    global _nc_cache
    if _nc_cache is None:
        _nc_cache = _build()
    return _nc_cache


def _pack_w12(w12_e):
    """[2816, 512] -> [128, 22*4*128] in PE consumption order.

    w12p[p, (t, kd, c)] = w12_e.T[kd*128 + p, col(t, c)] with
    col(2*hh, c) = hh*128 + c (gate), col(2*hh+1, c) = H + hh*128 + c (up).
    """
    a = np.ascontiguousarray(w12_e.T).astype(NP_F16)  # [D, 2H]
    a = a.reshape(KD, 128, 2, KH, 128)  # [kd, p, gu, hh, c]
    a = a.transpose(1, 3, 2, 0, 4)  # [p, hh, gu, kd, c]
    return np.ascontiguousarray(a.reshape(128, NTILE * KD * 128))


def kernel(sorted_x, w12, w3, expert_starts, expert_ends):
    global last_exec_time_ns, last_trace_path
    sorted_x = np.asarray(sorted_x)
    w12 = np.asarray(w12)
    w3 = np.asarray(w3)
    starts = np.asarray(expert_starts).astype(np.int64)
    T = sorted_x.shape[0]

    in_maps = []
    for e in range(N_CORES):
        # jax.lax.dynamic_slice clamps the start index the same way
        s = int(min(max(starts[e], 0), T - TPE))
        xe = sorted_x[s : s + TPE]  # (TPE, D) f32
        in_maps.append(
            {
                "xt": np.ascontiguousarray(xe.T).astype(NP_F16),
                "w12p": _pack_w12(w12[e]),
                "w3t": np.ascontiguousarray(w3[e].T).astype(NP_F16),
            }
        )

    trace = bool(os.environ.get("BASS_MOE_TRACE"))
    res = run_bass_kernel_spmd(
        _get_nc(), in_maps, core_ids=list(range(N_CORES)), trace=trace
    )
    if trace:
        last_exec_time_ns = res.exec_time_ns
        iat = res.instructions_and_trace
        last_trace_path = iat[1] if iat else None

    out = np.empty((N_CORES * TPE, D), dtype=np.float32)
    for e in range(N_CORES):
        out[e * TPE : (e + 1) * TPE] = res.results[e]["outT"].T.astype(np.float32)
    return out


# revision 21
# speedup vs baseline: 1.0042x; 1.0042x over previous
"""Grouped SwiGLU expert MLP (MoE) on 8 Trainium2 NeuronCores.

Problem: sorted_x [32768, 512] f32, tokens pre-sorted by expert into 8 equal
contiguous segments of 4096 tokens; per-expert SwiGLU MLP
    h12 = x_e @ w12[e].T          (4096, 2816)
    h   = silu(h12[:, :1408]) * h12[:, 1408:]
    out = h @ w3[e].T             (4096, 512)

Sharding: pure expert parallelism — core e owns expert e's weights and its
4096-token segment (sliced host-side from expert_starts), so no device-side
collectives are needed; the host concatenates the per-core outputs.

Device layout is feature-major throughout ("contraction dim on partitions"),
which makes both GEMMs transpose-free on chip:
    xt   = x_e.T   [512, 4096]  fp16
    w12p = w12.T   pre-tiled    fp16  (see below)
    w3t  = w3.T    [1408, 512]  fp16
    outT = out.T   [512, 4096]  fp16  (host transposes + upcasts back)
GEMM1 produces H12^T tiles [128h, Nt] (PSUM f32), SwiGLU runs on ACT+DVE into
fp16 H^T tiles, GEMM2 consumes them directly. fp16 operands run the PE at
1 cycle/row; accumulation is always f32 in PSUM.

The PE stream (1056 matmuls x 512 moving rows = 225.3us warm) is the hard
floor; everything else is edge management:
  * warm-up matmuls on a memset scratch tile run during the fixed ~6.6us
    NEFF prologue + first-DMA window, so HAM un-throttles (1.2->2.4 GHz)
    before the real stream begins and the PE never sits idle at the head;
  * w12 is repacked host-side into PE consumption order — 22 stationary
    tile-groups t=(g0,u0,g1,u1,...) of [kd][128 cols], 128KB each with 1KB
    contiguous runs per partition (column-sliced chunks of the plain
    transposed layout have 256B runs and run at ~4x lower DMA bandwidth,
    which starved the PE for ~10us at the head in earlier revisions);
  * input DMAs are split/ordered by consumption time across the two queues
    (sync=SP: w12 tile stream + output stores; gpsimd=SWDGE: x0 per-kd,
    then w3, then x1..7);
  * GEMM2 runs do-major after each block's hh loop (not LAG-interleaved),
    so acc[do] banks finish one at a time and the PSUM->SBUF copy + output
    DMA of each do overlaps the remaining matmul stream; copies split in
    half across ACT and DVE; the very last store is split across both DMA
    queues so only ~64KB trails the final matmul;
  * outputs store as fp16 (adds ~1e-4 rel err vs the 2e-2 budget).
"""

import os

import numpy as np
import ml_dtypes

import concourse.bass as bass
import concourse.mybir as mybir
import concourse.tile as tile
from concourse import bacc
from concourse.bass_utils import run_bass_kernel_spmd

N_CORES = 8
D = 512  # d_model
H = 1408  # hidden
TWOH = 2 * H
TPE = 4096  # tokens per expert
NT = 512  # token block (matmul moving free dim, one PSUM bank in f32)
KD = D // 128  # 4 contraction tiles over d
KH = H // 128  # 11 contraction tiles over h
NB = TPE // NT  # token blocks
NTILE = 2 * KH  # 22 stationary tile-groups of w12 in consumption order

F16 = mybir.dt.float16
F32 = mybir.dt.float32
NP_F16 = np.dtype(np.float16)

N_WARMUP = 38  # LDW+MM pairs on scratch data before the real stream

# Results of a traced run (test harness reads these).
last_exec_time_ns = None
last_trace_path = None


def _build():
    # Bacc (not plain Bass): its compile() pass pipeline legalizes sync
    # waits (>=2 waits per instruction are split into event-sem chains),
    # which this image's walrus requires.
    nc = bacc.Bacc("TRN2", target_bir_lowering=False, debug=False, num_devices=N_CORES)
    xt = nc.dram_tensor("xt", [D, TPE], F16, kind="ExternalInput")
    w12p = nc.dram_tensor("w12p", [128, NTILE * KD * 128], F16, kind="ExternalInput")
    w3t = nc.dram_tensor("w3t", [H, D], F16, kind="ExternalInput")
    outT = nc.dram_tensor("outT", [D, TPE], F16, kind="ExternalOutput")

    with tile.TileContext(nc) as tc:
        with (
            tc.tile_pool(name="weights", bufs=1) as wpool,
            tc.tile_pool(name="xin", bufs=1) as xpool,
            tc.tile_pool(name="ht", bufs=2) as hpool,
            tc.tile_pool(name="swi", bufs=4) as spool,
            tc.tile_pool(name="ot", bufs=6) as opool,
            tc.tile_pool(name="pg", bufs=2, space=bass.MemorySpace.PSUM) as pgate,
            tc.tile_pool(name="pu", bufs=2, space=bass.MemorySpace.PSUM) as pup,
            tc.tile_pool(name="po", bufs=1, space=bass.MemorySpace.PSUM) as pacc,
        ):
            # w12s[p, t, kd, c]: t = 2*hh (gate) / 2*hh+1 (up)
            w12s = wpool.tile([128, NTILE, KD, 128], F16)
            w3s = wpool.tile([128, KH, D], F16)
            xs = xpool.tile([128, KD, TPE], F16)
            warm = wpool.tile([128, 128], F16)

            xt_r = xt[:, :].rearrange("(kd p) t -> p kd t", p=128)
            w12_r = w12p[:, :].rearrange("p (t kd c) -> p t kd c", kd=KD, c=128)
            w3_r = w3t[:, :].rearrange("(kh p) d -> p kh d", p=128)
            outT_r = outT[:, :].rearrange("(do p) t -> p do t", p=128)

            # Warm-up scratch init on DVE (idle at start) so neither DMA
            # sequencer is delayed and the PE warm-up matmuls' dependency
            # resolves during the prologue.
            nc.vector.memset(warm[:], 0)

            # Each DMA_DIRECT2D costs ~650ns of sequencer issue time and the
            # two dynamic queues (sync=SP, gpsimd=Pool/SWDGE) split ~358GB/s
            # of HBM read bandwidth, so chunks are sized/ordered to land just
            # ahead of PE consumption: w12 tile t is consumed at roughly
            # first_mm + 0.87*t us, w3 at +19us, x block tb at +28.2*tb us.
            # Early aggregate DMA bandwidth is only ~250-300GB/s while both
            # queues ramp, and a lone 512KB x0 lands with ~1us of run-to-run
            # jitter — late enough that the idle after the warm-up matmuls
            # can trip HAM's free-running MID window and re-throttle the PE.
            # So x0 is split across BOTH queues as their first transfer
            # (~256KB each, consistent ~11.7us arrival), with the w12 tiles
            # interleaved across queues in consumption order behind it; w3
            # and x1..7 queue last (w3 isn't needed until block 0's GEMM2).
            def dma_w12(q, t0, t1):
                q.dma_start(out=w12s[:, t0:t1, :, :], in_=w12_r[:, t0:t1, :, :])

            nc.sync.dma_start(out=xs[:, 0:2, 0:NT], in_=xt_r[:, 0:2, 0:NT])
            nc.gpsimd.dma_start(out=xs[:, 2:KD, 0:NT], in_=xt_r[:, 2:KD, 0:NT])
            for t0, t1 in [(0, 1), (1, 2), (6, 10), (14, 18)]:
                dma_w12(nc.sync, t0, t1)
            for t0, t1 in [(2, 4), (4, 6), (10, 14), (18, NTILE)]:
                dma_w12(nc.gpsimd, t0, t1)
            nc.gpsimd.dma_start(out=w3s[:, 0:6, :], in_=w3_r[:, 0:6, :])
            nc.gpsimd.dma_start(out=w3s[:, 6:KH, :], in_=w3_r[:, 6:KH, :])
            for tb in range(1, NB):
                nc.gpsimd.dma_start(
                    out=xs[:, :, tb * NT : (tb + 1) * NT],
                    in_=xt_r[:, :, tb * NT : (tb + 1) * NT],
                )

            # Warm-up: PE activity with no DMA dependency, issued first so it
            # runs during the prologue/first-chunk window and flips HAM to
            # 8/8 before the real matmuls.  Garbage values, never read;
            # shares the ps_g tag so it cycles inside pgate's 2 banks.
            wps = pgate.tile([128, NT], F32, name="ps_g", tag="ps_g")
            for _ in range(N_WARMUP):
                nc.tensor.matmul(wps[:, 0:128], warm[:], warm[:], start=True, stop=True)

            for tb in range(NB):
                tsl = bass.ts(tb, NT)
                ht = hpool.tile([128, KH, NT], F16)
                acc = [
                    pacc.tile([128, NT], F32, name=f"acc{do}", tag=f"acc{do}")
                    for do in range(KD)
                ]

                # GEMM1 + SwiGLU, hh-major; gate before up, matching the
                # arrival order of the w12 tile stream (t_even = gate).
                for hh in range(KH):
                    ps_g = pgate.tile([128, NT], F32)
                    ps_u = pup.tile([128, NT], F32)
                    for kd in range(KD):
                        nc.tensor.matmul(
                            ps_g[:],
                            w12s[:, 2 * hh, kd, :],
                            xs[:, kd, tsl],
                            start=(kd == 0),
                            stop=(kd == KD - 1),
                        )
                    for kd in range(KD):
                        nc.tensor.matmul(
                            ps_u[:],
                            w12s[:, 2 * hh + 1, kd, :],
                            xs[:, kd, tsl],
                            start=(kd == 0),
                            stop=(kd == KD - 1),
                        )
                    sil = spool.tile([128, NT], F32)
                    nc.scalar.activation(
                        sil[:], ps_g[:], mybir.ActivationFunctionType.Silu
                    )
                    nc.vector.tensor_mul(ht[:, hh, :], sil[:], ps_u[:])

                # GEMM2 do-major: each acc bank finishes 11 matmuls before the
                # next starts, so its copy+store overlap the remaining stream.
                # The kh chain ends at kh=10 whose ht lands ~1.4us after the
                # last GEMM1 matmul — covered by the 10 preceding matmuls.
                for do in range(KD):
                    last = tb == NB - 1 and do == KD - 1
                    ot = opool.tile([128, NT], F16)
                    hn = NT // 2
                    t0 = tb * NT
                    if not last:
                        for kh in range(KH):
                            nc.tensor.matmul(
                                acc[do][:],
                                w3s[:, kh, do * 128 : (do + 1) * 128],
                                ht[:, kh, :],
                                start=(kh == 0),
                                stop=(kh == KH - 1),
                            )
                        # PSUM copies on one engine each (bass serializes
                        # same-bank ACT+DVE access), alternating engines
                        if do % 2 == 0:
                            nc.scalar.copy(ot[:], acc[do][:])
                        else:
                            nc.vector.tensor_copy(ot[:], acc[do][:])
                        nc.sync.dma_start(out=outT_r[:, do, tsl], in_=ot[:])
                    else:
                        # the very last chain runs as two 256-token halves so
                        # the first half's copy+store overlap the second
                        # half's matmuls and only ~32KB trails the final MM.
                        # Half B accumulates in acc[0]'s bank (copied out 33
                        # matmuls ago) — PE may not write a bank a copy is
                        # still reading.  Both halves store via the sync
                        # queue — it has been streaming stores all along; the
                        # gpsimd queue is cold and costs ~1us to re-ramp.
                        for (lo, hi), pt, eng in (
                            ((0, hn), acc[do], nc.scalar),
                            ((hn, NT), acc[0], nc.vector),
                        ):
                            for kh in range(KH):
                                nc.tensor.matmul(
                                    pt[:, 0:hn],
                                    w3s[:, kh, do * 128 : (do + 1) * 128],
                                    ht[:, kh, lo:hi],
                                    start=(kh == 0),
                                    stop=(kh == KH - 1),
                                )
                            if eng is nc.scalar:
                                eng.copy(ot[:, lo:hi], pt[:, 0:hn])
                            else:
                                eng.tensor_copy(ot[:, lo:hi], pt[:, 0:hn])
                            nc.sync.dma_start(
                                out=outT_r[:, do, t0 + lo : t0 + hi],
                                in_=ot[:, lo:hi],
                            )
    nc.compile()
    return nc


_nc_cache = None


def _get_nc():
    global _nc_cache
    if _nc_cache is None:
        _nc_cache = _build()
    return _nc_cache


def _pack_w12(w12_e):
    """[2816, 512] -> [128, 22*4*128] in PE consumption order.

    w12p[p, (t, kd, c)] = w12_e.T[kd*128 + p, col(t, c)] with
    col(2*hh, c) = hh*128 + c (gate), col(2*hh+1, c) = H + hh*128 + c (up).
    """
    a = np.ascontiguousarray(w12_e.T).astype(NP_F16)  # [D, 2H]
    a = a.reshape(KD, 128, 2, KH, 128)  # [kd, p, gu, hh, c]
    a = a.transpose(1, 3, 2, 0, 4)  # [p, hh, gu, kd, c]
    return np.ascontiguousarray(a.reshape(128, NTILE * KD * 128))


def kernel(sorted_x, w12, w3, expert_starts, expert_ends):
    global last_exec_time_ns, last_trace_path
    sorted_x = np.asarray(sorted_x)
    w12 = np.asarray(w12)
    w3 = np.asarray(w3)
    starts = np.asarray(expert_starts).astype(np.int64)
    T = sorted_x.shape[0]

    in_maps = []
    for e in range(N_CORES):
        # jax.lax.dynamic_slice clamps the start index the same way
        s = int(min(max(starts[e], 0), T - TPE))
        xe = sorted_x[s : s + TPE]  # (TPE, D) f32
        in_maps.append(
            {
                "xt": np.ascontiguousarray(xe.T).astype(NP_F16),
                "w12p": _pack_w12(w12[e]),
                "w3t": np.ascontiguousarray(w3[e].T).astype(NP_F16),
            }
        )

    trace = bool(os.environ.get("BASS_MOE_TRACE"))
    res = run_bass_kernel_spmd(
        _get_nc(), in_maps, core_ids=list(range(N_CORES)), trace=trace
    )
    if trace:
        last_exec_time_ns = res.exec_time_ns
        iat = res.instructions_and_trace
        last_trace_path = iat[1] if iat else None

    out = np.empty((N_CORES * TPE, D), dtype=np.float32)
    for e in range(N_CORES):
        out[e * TPE : (e + 1) * TPE] = res.results[e]["outT"].T.astype(np.float32)
    return out


# revision 24
# speedup vs baseline: 1.0063x; 1.0021x over previous
"""Grouped SwiGLU expert MLP (MoE) on 8 Trainium2 NeuronCores.

Problem: sorted_x [32768, 512] f32, tokens pre-sorted by expert into 8 equal
contiguous segments of 4096 tokens; per-expert SwiGLU MLP
    h12 = x_e @ w12[e].T          (4096, 2816)
    h   = silu(h12[:, :1408]) * h12[:, 1408:]
    out = h @ w3[e].T             (4096, 512)

Sharding: pure expert parallelism — core e owns expert e's weights and its
4096-token segment (sliced host-side from expert_starts), so no device-side
collectives are needed; the host concatenates the per-core outputs.

Device layout is feature-major throughout ("contraction dim on partitions"),
which makes both GEMMs transpose-free on chip:
    xt   = x_e.T   [512, 4096]  fp16
    w12p = w12.T   pre-tiled    fp16  (see below)
    w3t  = w3.T    [1408, 512]  fp16
    outT = out.T   [512, 4096]  fp16  (host transposes + upcasts back)
GEMM1 produces H12^T tiles [128h, Nt] (PSUM f32), SwiGLU runs on ACT+DVE into
fp16 H^T tiles, GEMM2 consumes them directly. fp16 operands run the PE at
1 cycle/row; accumulation is always f32 in PSUM.

The PE stream (1056 matmuls x 512 moving rows = 225.3us warm) is the hard
floor; everything else is edge management:
  * warm-up matmuls on a memset scratch tile run during the fixed ~6.6us
    NEFF prologue + first-DMA window, so HAM un-throttles (1.2->2.4 GHz)
    before the real stream begins and the PE never sits idle at the head;
  * w12 is repacked host-side into PE consumption order — 22 stationary
    tile-groups t=(g0,u0,g1,u1,...) of [kd][128 cols], 128KB each with 1KB
    contiguous runs per partition (column-sliced chunks of the plain
    transposed layout have 256B runs and run at ~4x lower DMA bandwidth,
    which starved the PE for ~10us at the head in earlier revisions);
  * input DMAs are split/ordered by consumption time across the two queues
    (sync=SP: w12 tile stream + output stores; gpsimd=SWDGE: x0 per-kd,
    then w3, then x1..7);
  * GEMM2 runs do-major after each block's hh loop (not LAG-interleaved),
    so acc[do] banks finish one at a time and the PSUM->SBUF copy + output
    DMA of each do overlaps the remaining matmul stream; copies split in
    half across ACT and DVE; the very last store is split across both DMA
    queues so only ~64KB trails the final matmul;
  * outputs store as fp16 (adds ~1e-4 rel err vs the 2e-2 budget).
"""

import os

import numpy as np
import ml_dtypes

import concourse.bass as bass
import concourse.mybir as mybir
import concourse.tile as tile
from concourse import bacc
from concourse.bass_utils import run_bass_kernel_spmd

N_CORES = 8
D = 512  # d_model
H = 1408  # hidden
TWOH = 2 * H
TPE = 4096  # tokens per expert
NT = 512  # token block (matmul moving free dim, one PSUM bank in f32)
KD = D // 128  # 4 contraction tiles over d
KH = H // 128  # 11 contraction tiles over h
NB = TPE // NT  # token blocks
NTILE = 2 * KH  # 22 stationary tile-groups of w12 in consumption order

F16 = mybir.dt.float16
F32 = mybir.dt.float32
NP_F16 = np.dtype(np.float16)

N_WARMUP = 52  # LDW+MM pairs on scratch data before the real stream

# Results of a traced run (test harness reads these).
last_exec_time_ns = None
last_trace_path = None


def _build():
    # Bacc (not plain Bass): its compile() pass pipeline legalizes sync
    # waits (>=2 waits per instruction are split into event-sem chains),
    # which this image's walrus requires.
    nc = bacc.Bacc("TRN2", target_bir_lowering=False, debug=False, num_devices=N_CORES)
    xt = nc.dram_tensor("xt", [D, TPE], F16, kind="ExternalInput")
    w12p = nc.dram_tensor("w12p", [128, NTILE * KD * 128], F16, kind="ExternalInput")
    w3t = nc.dram_tensor("w3t", [H, D], F16, kind="ExternalInput")
    outT = nc.dram_tensor("outT", [D, TPE], F16, kind="ExternalOutput")

    with tile.TileContext(nc) as tc:
        with (
            tc.tile_pool(name="weights", bufs=1) as wpool,
            tc.tile_pool(name="xin", bufs=1) as xpool,
            tc.tile_pool(name="ht", bufs=2) as hpool,
            tc.tile_pool(name="swi", bufs=4) as spool,
            tc.tile_pool(name="ot", bufs=6) as opool,
            tc.tile_pool(name="pg", bufs=2, space=bass.MemorySpace.PSUM) as pgate,
            tc.tile_pool(name="pu", bufs=2, space=bass.MemorySpace.PSUM) as pup,
            tc.tile_pool(name="po", bufs=1, space=bass.MemorySpace.PSUM) as pacc,
        ):
            # w12s[p, t, kd, c]: t = 2*hh (gate) / 2*hh+1 (up)
            w12s = wpool.tile([128, NTILE, KD, 128], F16)
            w3s = wpool.tile([128, KH, D], F16)
            xs = xpool.tile([128, KD, TPE], F16)
            warm = wpool.tile([128, 128], F16)

            xt_r = xt[:, :].rearrange("(kd p) t -> p kd t", p=128)
            w12_r = w12p[:, :].rearrange("p (t kd c) -> p t kd c", kd=KD, c=128)
            w3_r = w3t[:, :].rearrange("(kh p) d -> p kh d", p=128)
            outT_r = outT[:, :].rearrange("(do p) t -> p do t", p=128)

            # Warm-up scratch init on DVE (idle at start) so neither DMA
            # sequencer is delayed and the PE warm-up matmuls' dependency
            # resolves during the prologue.
            nc.vector.memset(warm[:], 0)

            # Each DMA_DIRECT2D costs ~650ns of sequencer issue time and the
            # two dynamic queues (sync=SP, gpsimd=Pool/SWDGE) split ~358GB/s
            # of HBM read bandwidth, so chunks are sized/ordered to land just
            # ahead of PE consumption: w12 tile t is consumed at roughly
            # first_mm + 0.87*t us, w3 at +19us, x block tb at +28.2*tb us.
            # x0 as ONE 512KB transfer on gpsimd, alone until it completes
            # (early aggregate DMA bw is only ~250-300GB/s while both queues
            # ramp; anything sharing the queue delays the first real matmul;
            # x0 lands at ~13.1us +/- 1us of run-to-run jitter and the
            # warm-up matmul count is sized to cover that window — a PE idle
            # >1.7us here can trip HAM's free-running MID window and
            # re-throttle the PE to 1.2GHz for 3.4us).  w12 tiles split
            # across BOTH queues in consumption order; w3 and x1..7 queue
            # behind (w3 isn't needed until block 0's GEMM2).
            def dma_w12(q, t0, t1):
                q.dma_start(out=w12s[:, t0:t1, :, :], in_=w12_r[:, t0:t1, :, :])

            nc.gpsimd.dma_start(out=xs[:, :, 0:NT], in_=xt_r[:, :, 0:NT])
            for t0, t1 in [(0, 1), (1, 2), (2, 4), (6, 10), (14, 18)]:
                dma_w12(nc.sync, t0, t1)
            for t0, t1 in [(4, 6), (10, 14), (18, NTILE)]:
                dma_w12(nc.gpsimd, t0, t1)
            nc.gpsimd.dma_start(out=w3s[:, 0:6, :], in_=w3_r[:, 0:6, :])
            nc.gpsimd.dma_start(out=w3s[:, 6:KH, :], in_=w3_r[:, 6:KH, :])
            for tb in range(1, NB):
                nc.gpsimd.dma_start(
                    out=xs[:, :, tb * NT : (tb + 1) * NT],
                    in_=xt_r[:, :, tb * NT : (tb + 1) * NT],
                )

            # Warm-up: PE activity with no DMA dependency, issued first so it
            # runs during the prologue/first-chunk window and flips HAM to
            # 8/8 before the real matmuls.  Garbage values, never read;
            # shares the ps_g tag so it cycles inside pgate's 2 banks.
            wps = pgate.tile([128, NT], F32, name="ps_g", tag="ps_g")
            for _ in range(N_WARMUP):
                nc.tensor.matmul(wps[:, 0:128], warm[:], warm[:], start=True, stop=True)

            for tb in range(NB):
                tsl = bass.ts(tb, NT)
                ht = hpool.tile([128, KH, NT], F16)
                acc = [
                    pacc.tile([128, NT], F32, name=f"acc{do}", tag=f"acc{do}")
                    for do in range(KD)
                ]

                # GEMM1 + SwiGLU, hh-major, gate/up interleaved per kd
                for hh in range(KH):
                    ps_g = pgate.tile([128, NT], F32)
                    ps_u = pup.tile([128, NT], F32)
                    for kd in range(KD):
                        nc.tensor.matmul(
                            ps_g[:],
                            w12s[:, 2 * hh, kd, :],
                            xs[:, kd, tsl],
                            start=(kd == 0),
                            stop=(kd == KD - 1),
                        )
                        nc.tensor.matmul(
                            ps_u[:],
                            w12s[:, 2 * hh + 1, kd, :],
                            xs[:, kd, tsl],
                            start=(kd == 0),
                            stop=(kd == KD - 1),
                        )
                    sil = spool.tile([128, NT], F32)
                    nc.scalar.activation(
                        sil[:], ps_g[:], mybir.ActivationFunctionType.Silu
                    )
                    nc.vector.tensor_mul(ht[:, hh, :], sil[:], ps_u[:])

                # GEMM2 do-major: each acc bank finishes 11 matmuls before the
                # next starts, so its copy+store overlap the remaining stream.
                # The kh chain ends at kh=10 whose ht lands ~1.4us after the
                # last GEMM1 matmul — covered by the 10 preceding matmuls.
                for do in range(KD):
                    last = tb == NB - 1 and do == KD - 1
                    ot = opool.tile([128, NT], F16)
                    hn = NT // 2
                    t0 = tb * NT
                    if not last:
                        for kh in range(KH):
                            nc.tensor.matmul(
                                acc[do][:],
                                w3s[:, kh, do * 128 : (do + 1) * 128],
                                ht[:, kh, :],
                                start=(kh == 0),
                                stop=(kh == KH - 1),
                            )
                        # PSUM copies on one engine each (bass serializes
                        # same-bank ACT+DVE access), alternating engines
                        if do % 2 == 0:
                            nc.scalar.copy(ot[:], acc[do][:])
                        else:
                            nc.vector.tensor_copy(ot[:], acc[do][:])
                        nc.sync.dma_start(out=outT_r[:, do, tsl], in_=ot[:])
                    else:
                        # the very last chain runs as two 256-token halves so
                        # the first half's copy+store overlap the second
                        # half's matmuls and only ~32KB trails the final MM.
                        # Half B accumulates in acc[0]'s bank (copied out 33
                        # matmuls ago) — PE may not write a bank a copy is
                        # still reading.  Both halves store via the sync
                        # queue — it has been streaming stores all along; the
                        # gpsimd queue is cold and costs ~1us to re-ramp.
                        for (lo, hi), pt, eng in (
                            ((0, hn), acc[do], nc.scalar),
                            ((hn, NT), acc[0], nc.vector),
                        ):
                            for kh in range(KH):
                                nc.tensor.matmul(
                                    pt[:, 0:hn],
                                    w3s[:, kh, do * 128 : (do + 1) * 128],
                                    ht[:, kh, lo:hi],
                                    start=(kh == 0),
                                    stop=(kh == KH - 1),
                                )
                            if eng is nc.scalar:
                                eng.copy(ot[:, lo:hi], pt[:, 0:hn])
                            else:
                                eng.tensor_copy(ot[:, lo:hi], pt[:, 0:hn])
                            nc.sync.dma_start(
                                out=outT_r[:, do, t0 + lo : t0 + hi],
                                in_=ot[:, lo:hi],
                            )
    nc.compile()
    return nc


_nc_cache = None


def _get_nc():
    global _nc_cache
    if _nc_cache is None:
        _nc_cache = _build()
    return _nc_cache


def _pack_w12(w12_e):
    """[2816, 512] -> [128, 22*4*128] in PE consumption order.

    w12p[p, (t, kd, c)] = w12_e.T[kd*128 + p, col(t, c)] with
    col(2*hh, c) = hh*128 + c (gate), col(2*hh+1, c) = H + hh*128 + c (up).
    """
    a = np.ascontiguousarray(w12_e.T).astype(NP_F16)  # [D, 2H]
    a = a.reshape(KD, 128, 2, KH, 128)  # [kd, p, gu, hh, c]
    a = a.transpose(1, 3, 2, 0, 4)  # [p, hh, gu, kd, c]
    return np.ascontiguousarray(a.reshape(128, NTILE * KD * 128))


def kernel(sorted_x, w12, w3, expert_starts, expert_ends):
    global last_exec_time_ns, last_trace_path
    sorted_x = np.asarray(sorted_x)
    w12 = np.asarray(w12)
    w3 = np.asarray(w3)
    starts = np.asarray(expert_starts).astype(np.int64)
    T = sorted_x.shape[0]

    in_maps = []
    for e in range(N_CORES):
        # jax.lax.dynamic_slice clamps the start index the same way
        s = int(min(max(starts[e], 0), T - TPE))
        xe = sorted_x[s : s + TPE]  # (TPE, D) f32
        in_maps.append(
            {
                "xt": np.ascontiguousarray(xe.T).astype(NP_F16),
                "w12p": _pack_w12(w12[e]),
                "w3t": np.ascontiguousarray(w3[e].T).astype(NP_F16),
            }
        )

    trace = bool(os.environ.get("BASS_MOE_TRACE"))
    res = run_bass_kernel_spmd(
        _get_nc(), in_maps, core_ids=list(range(N_CORES)), trace=trace
    )
    if trace:
        last_exec_time_ns = res.exec_time_ns
        iat = res.instructions_and_trace
        last_trace_path = iat[1] if iat else None

    out = np.empty((N_CORES * TPE, D), dtype=np.float32)
    for e in range(N_CORES):
        out[e * TPE : (e + 1) * TPE] = res.results[e]["outT"].T.astype(np.float32)
    return out


# revision 25
# speedup vs baseline: 1.0090x; 1.0027x over previous
"""Grouped SwiGLU expert MLP (MoE) on 8 Trainium2 NeuronCores.

Problem: sorted_x [32768, 512] f32, tokens pre-sorted by expert into 8 equal
contiguous segments of 4096 tokens; per-expert SwiGLU MLP
    h12 = x_e @ w12[e].T          (4096, 2816)
    h   = silu(h12[:, :1408]) * h12[:, 1408:]
    out = h @ w3[e].T             (4096, 512)

Sharding: pure expert parallelism — core e owns expert e's weights and its
4096-token segment (sliced host-side from expert_starts), so no device-side
collectives are needed; the host concatenates the per-core outputs.

Device layout is feature-major throughout ("contraction dim on partitions"),
which makes both GEMMs transpose-free on chip:
    xt   = x_e.T   [512, 4096]  fp16
    w12p = w12.T   pre-tiled    fp16  (see below)
    w3t  = w3.T    [1408, 512]  fp16
    outT = out.T   [512, 4096]  fp16  (host transposes + upcasts back)
GEMM1 produces H12^T tiles [128h, Nt] (PSUM f32), SwiGLU runs on ACT+DVE into
fp16 H^T tiles, GEMM2 consumes them directly. fp16 operands run the PE at
1 cycle/row; accumulation is always f32 in PSUM.

The PE stream (1056 matmuls x 512 moving rows = 225.3us warm) is the hard
floor; everything else is edge management:
  * warm-up matmuls on a memset scratch tile run during the fixed ~6.6us
    NEFF prologue + first-DMA window, so HAM un-throttles (1.2->2.4 GHz)
    before the real stream begins and the PE never sits idle at the head;
  * w12 is repacked host-side into PE consumption order — 22 stationary
    tile-groups t=(g0,u0,g1,u1,...) of [kd][128 cols], 128KB each with 1KB
    contiguous runs per partition (column-sliced chunks of the plain
    transposed layout have 256B runs and run at ~4x lower DMA bandwidth,
    which starved the PE for ~10us at the head in earlier revisions);
  * input DMAs are split/ordered by consumption time across the two queues
    (sync=SP: w12 tile stream + output stores; gpsimd=SWDGE: x0 per-kd,
    then w3, then x1..7);
  * GEMM2 runs do-major after each block's hh loop (not LAG-interleaved),
    so acc[do] banks finish one at a time and the PSUM->SBUF copy + output
    DMA of each do overlaps the remaining matmul stream; copies split in
    half across ACT and DVE; the very last store is split across both DMA
    queues so only ~64KB trails the final matmul;
  * outputs store as fp16 (adds ~1e-4 rel err vs the 2e-2 budget).
"""

import os

import numpy as np
import ml_dtypes

import concourse.bass as bass
import concourse.mybir as mybir
import concourse.tile as tile
from concourse import bacc
from concourse.bass_utils import run_bass_kernel_spmd

N_CORES = 8
D = 512  # d_model
H = 1408  # hidden
TWOH = 2 * H
TPE = 4096  # tokens per expert
NT = 512  # token block (matmul moving free dim, one PSUM bank in f32)
KD = D // 128  # 4 contraction tiles over d
KH = H // 128  # 11 contraction tiles over h
NB = TPE // NT  # token blocks
NTILE = 2 * KH  # 22 stationary tile-groups of w12 in consumption order

F16 = mybir.dt.float16
F32 = mybir.dt.float32
NP_F16 = np.dtype(np.float16)

N_WARMUP = 55  # LDW+MM pairs on scratch data before the real stream

# Results of a traced run (test harness reads these).
last_exec_time_ns = None
last_trace_path = None


def _build():
    # Bacc (not plain Bass): its compile() pass pipeline legalizes sync
    # waits (>=2 waits per instruction are split into event-sem chains),
    # which this image's walrus requires.
    nc = bacc.Bacc("TRN2", target_bir_lowering=False, debug=False, num_devices=N_CORES)
    xt = nc.dram_tensor("xt", [D, TPE], F16, kind="ExternalInput")
    w12p = nc.dram_tensor("w12p", [128, NTILE * KD * 128], F16, kind="ExternalInput")
    w3t = nc.dram_tensor("w3t", [H, D], F16, kind="ExternalInput")
    outT = nc.dram_tensor("outT", [D, TPE], F16, kind="ExternalOutput")

    with tile.TileContext(nc) as tc:
        with (
            tc.tile_pool(name="weights", bufs=1) as wpool,
            tc.tile_pool(name="xin", bufs=1) as xpool,
            tc.tile_pool(name="ht", bufs=2) as hpool,
            tc.tile_pool(name="swi", bufs=4) as spool,
            tc.tile_pool(name="ot", bufs=6) as opool,
            tc.tile_pool(name="pg", bufs=2, space=bass.MemorySpace.PSUM) as pgate,
            tc.tile_pool(name="pu", bufs=2, space=bass.MemorySpace.PSUM) as pup,
            tc.tile_pool(name="po", bufs=1, space=bass.MemorySpace.PSUM) as pacc,
        ):
            # w12s[p, t, kd, c]: t = 2*hh (gate) / 2*hh+1 (up)
            w12s = wpool.tile([128, NTILE, KD, 128], F16)
            w3s = wpool.tile([128, KH, D], F16)
            xs = xpool.tile([128, KD, TPE], F16)
            warm = wpool.tile([128, 128], F16)

            xt_r = xt[:, :].rearrange("(kd p) t -> p kd t", p=128)
            w12_r = w12p[:, :].rearrange("p (t kd c) -> p t kd c", kd=KD, c=128)
            w3_r = w3t[:, :].rearrange("(kh p) d -> p kh d", p=128)
            outT_r = outT[:, :].rearrange("(do p) t -> p do t", p=128)

            # Warm-up scratch init on DVE (idle at start) so neither DMA
            # sequencer is delayed and the PE warm-up matmuls' dependency
            # resolves during the prologue.
            nc.vector.memset(warm[:], 0)

            # Each DMA_DIRECT2D costs ~650ns of sequencer issue time and the
            # two dynamic queues (sync=SP, gpsimd=Pool/SWDGE) split ~358GB/s
            # of HBM read bandwidth, so chunks are sized/ordered to land just
            # ahead of PE consumption: w12 tile t is consumed at roughly
            # first_mm + 0.87*t us, w3 at +19us, x block tb at +28.2*tb us.
            # x0 as ONE 512KB transfer on gpsimd, alone until it completes
            # (early aggregate DMA bw is only ~250-300GB/s while both queues
            # ramp; anything sharing the queue delays the first real matmul;
            # x0 lands at ~13.1us +/- 1us of run-to-run jitter and the
            # warm-up matmul count is sized to cover that window — a PE idle
            # >1.7us here can trip HAM's free-running MID window and
            # re-throttle the PE to 1.2GHz for 3.4us).  w12 tiles split
            # across BOTH queues in consumption order; w3 and x1..7 queue
            # behind (w3 isn't needed until block 0's GEMM2).
            def dma_w12(q, t0, t1):
                q.dma_start(out=w12s[:, t0:t1, :, :], in_=w12_r[:, t0:t1, :, :])

            nc.gpsimd.dma_start(out=xs[:, :, 0:NT], in_=xt_r[:, :, 0:NT])
            for t0, t1 in [(0, 1), (1, 2), (2, 4), (6, 10), (14, 18)]:
                dma_w12(nc.sync, t0, t1)
            for t0, t1 in [(4, 6), (10, 14), (18, NTILE)]:
                dma_w12(nc.gpsimd, t0, t1)
            nc.gpsimd.dma_start(out=w3s[:, 0:6, :], in_=w3_r[:, 0:6, :])
            nc.gpsimd.dma_start(out=w3s[:, 6:KH, :], in_=w3_r[:, 6:KH, :])
            for tb in range(1, NB):
                nc.gpsimd.dma_start(
                    out=xs[:, :, tb * NT : (tb + 1) * NT],
                    in_=xt_r[:, :, tb * NT : (tb + 1) * NT],
                )

            # Warm-up: PE activity with no DMA dependency, issued first so it
            # runs during the prologue/first-chunk window and flips HAM to
            # 8/8 before the real matmuls.  Garbage values, never read;
            # shares the ps_g tag so it cycles inside pgate's 2 banks.
            wps = pgate.tile([128, NT], F32, name="ps_g", tag="ps_g")
            for _ in range(N_WARMUP):
                nc.tensor.matmul(wps[:, 0:128], warm[:], warm[:], start=True, stop=True)

            for tb in range(NB):
                tsl = bass.ts(tb, NT)
                ht = hpool.tile([128, KH, NT], F16)
                acc = [
                    pacc.tile([128, NT], F32, name=f"acc{do}", tag=f"acc{do}")
                    for do in range(KD)
                ]

                # GEMM1 + SwiGLU, hh-major, gate/up interleaved per kd
                for hh in range(KH):
                    ps_g = pgate.tile([128, NT], F32)
                    ps_u = pup.tile([128, NT], F32)
                    for kd in range(KD):
                        nc.tensor.matmul(
                            ps_g[:],
                            w12s[:, 2 * hh, kd, :],
                            xs[:, kd, tsl],
                            start=(kd == 0),
                            stop=(kd == KD - 1),
                        )
                        nc.tensor.matmul(
                            ps_u[:],
                            w12s[:, 2 * hh + 1, kd, :],
                            xs[:, kd, tsl],
                            start=(kd == 0),
                            stop=(kd == KD - 1),
                        )
                    sil = spool.tile([128, NT], F32)
                    nc.scalar.activation(
                        sil[:], ps_g[:], mybir.ActivationFunctionType.Silu
                    )
                    nc.vector.tensor_mul(ht[:, hh, :], sil[:], ps_u[:])

                # GEMM2 do-major: each acc bank finishes 11 matmuls before the
                # next starts, so its copy+store overlap the remaining stream.
                # The kh chain ends at kh=10 whose ht lands ~1.4us after the
                # last GEMM1 matmul — covered by the 10 preceding matmuls.
                for do in range(KD):
                    last = tb == NB - 1 and do == KD - 1
                    ot = opool.tile([128, NT], F16)
                    hn = NT // 2
                    t0 = tb * NT
                    if not last:
                        for kh in range(KH):
                            nc.tensor.matmul(
                                acc[do][:],
                                w3s[:, kh, do * 128 : (do + 1) * 128],
                                ht[:, kh, :],
                                start=(kh == 0),
                                stop=(kh == KH - 1),
                            )
                        # PSUM copies on one engine each (bass serializes
                        # same-bank ACT+DVE access), alternating engines
                        if do % 2 == 0:
                            nc.scalar.copy(ot[:], acc[do][:])
                        else:
                            nc.vector.tensor_copy(ot[:], acc[do][:])
                        nc.sync.dma_start(out=outT_r[:, do, tsl], in_=ot[:])
                    else:
                        # the very last chain runs as two 256-token halves so
                        # the first half's copy+store overlap the second
                        # half's matmuls and only ~32KB trails the final MM.
                        # Half B accumulates in acc[0]'s bank (copied out 33
                        # matmuls ago) — PE may not write a bank a copy is
                        # still reading.  Both halves store via the sync
                        # queue — it has been streaming stores all along; the
                        # gpsimd queue is cold and costs ~1us to re-ramp.
                        for (lo, hi), pt, eng in (
                            ((0, hn), acc[do], nc.scalar),
                            ((hn, NT), acc[0], nc.vector),
                        ):
                            for kh in range(KH):
                                nc.tensor.matmul(
                                    pt[:, 0:hn],
                                    w3s[:, kh, do * 128 : (do + 1) * 128],
                                    ht[:, kh, lo:hi],
                                    start=(kh == 0),
                                    stop=(kh == KH - 1),
                                )
                            if eng is nc.scalar:
                                eng.copy(ot[:, lo:hi], pt[:, 0:hn])
                            else:
                                eng.tensor_copy(ot[:, lo:hi], pt[:, 0:hn])
                            nc.sync.dma_start(
                                out=outT_r[:, do, t0 + lo : t0 + hi],
                                in_=ot[:, lo:hi],
                            )
    nc.compile()
    return nc


_nc_cache = None


def _get_nc():
    global _nc_cache
    if _nc_cache is None:
        _nc_cache = _build()
    return _nc_cache


def _pack_w12(w12_e):
    """[2816, 512] -> [128, 22*4*128] in PE consumption order.

    w12p[p, (t, kd, c)] = w12_e.T[kd*128 + p, col(t, c)] with
    col(2*hh, c) = hh*128 + c (gate), col(2*hh+1, c) = H + hh*128 + c (up).
    """
    a = np.ascontiguousarray(w12_e.T).astype(NP_F16)  # [D, 2H]
    a = a.reshape(KD, 128, 2, KH, 128)  # [kd, p, gu, hh, c]
    a = a.transpose(1, 3, 2, 0, 4)  # [p, hh, gu, kd, c]
    return np.ascontiguousarray(a.reshape(128, NTILE * KD * 128))


def kernel(sorted_x, w12, w3, expert_starts, expert_ends):
    global last_exec_time_ns, last_trace_path
    sorted_x = np.asarray(sorted_x)
    w12 = np.asarray(w12)
    w3 = np.asarray(w3)
    starts = np.asarray(expert_starts).astype(np.int64)
    T = sorted_x.shape[0]

    in_maps = []
    for e in range(N_CORES):
        # jax.lax.dynamic_slice clamps the start index the same way
        s = int(min(max(starts[e], 0), T - TPE))
        xe = sorted_x[s : s + TPE]  # (TPE, D) f32
        in_maps.append(
            {
                "xt": np.ascontiguousarray(xe.T).astype(NP_F16),
                "w12p": _pack_w12(w12[e]),
                "w3t": np.ascontiguousarray(w3[e].T).astype(NP_F16),
            }
        )

    trace = bool(os.environ.get("BASS_MOE_TRACE"))
    res = run_bass_kernel_spmd(
        _get_nc(), in_maps, core_ids=list(range(N_CORES)), trace=trace
    )
    if trace:
        last_exec_time_ns = res.exec_time_ns
        iat = res.instructions_and_trace
        last_trace_path = iat[1] if iat else None

    out = np.empty((N_CORES * TPE, D), dtype=np.float32)
    for e in range(N_CORES):
        out[e * TPE : (e + 1) * TPE] = res.results[e]["outT"].T.astype(np.float32)
    return out


# revision 31
# speedup vs baseline: 1.0090x; 1.0000x over previous
"""Grouped SwiGLU expert MLP (MoE) on 8 Trainium2 NeuronCores.

Problem: sorted_x [32768, 512] f32, tokens pre-sorted by expert into 8 equal
contiguous segments of 4096 tokens; per-expert SwiGLU MLP
    h12 = x_e @ w12[e].T          (4096, 2816)
    h   = silu(h12[:, :1408]) * h12[:, 1408:]
    out = h @ w3[e].T             (4096, 512)

Sharding: pure expert parallelism — core e owns expert e's weights and its
4096-token segment (sliced host-side from expert_starts), so no device-side
collectives are needed; the host concatenates the per-core outputs.

Device layout is feature-major throughout ("contraction dim on partitions"),
which makes both GEMMs transpose-free on chip:
    xt   = x_e.T   [512, 4096]  fp16
    w12p = w12.T   pre-tiled    fp16  (see below)
    w3t  = w3.T    [1408, 512]  fp16
    outT = out.T   [512, 4096]  fp16  (host transposes + upcasts back)
GEMM1 produces H12^T tiles [128h, Nt] (PSUM f32), SwiGLU runs on ACT+DVE into
fp16 H^T tiles, GEMM2 consumes them directly. fp16 operands run the PE at
1 cycle/row; accumulation is always f32 in PSUM.

The PE stream (1056 matmuls x 512 moving rows = 225.3us warm) is the hard
floor; everything else is edge management:
  * warm-up matmuls on a memset scratch tile run during the fixed ~6.6us
    NEFF prologue + first-DMA window, so HAM un-throttles (1.2->2.4 GHz)
    before the real stream begins and the PE never sits idle at the head;
  * w12 is repacked host-side into PE consumption order — 22 stationary
    tile-groups t=(g0,u0,g1,u1,...) of [kd][128 cols], 128KB each with 1KB
    contiguous runs per partition (column-sliced chunks of the plain
    transposed layout have 256B runs and run at ~4x lower DMA bandwidth,
    which starved the PE for ~10us at the head in earlier revisions);
  * input DMAs are split/ordered by consumption time across the two queues
    (sync=SP: w12 tile stream + output stores; gpsimd=SWDGE: x0 per-kd,
    then w3, then x1..7);
  * GEMM2 runs do-major after each block's hh loop (not LAG-interleaved),
    so acc[do] banks finish one at a time and the PSUM->SBUF copy + output
    DMA of each do overlaps the remaining matmul stream; copies split in
    half across ACT and DVE; the very last store is split across both DMA
    queues so only ~64KB trails the final matmul;
  * outputs store as fp16 (adds ~1e-4 rel err vs the 2e-2 budget).
"""

import os

import numpy as np
import ml_dtypes

import concourse.bass as bass
import concourse.mybir as mybir
import concourse.tile as tile
from concourse import bacc
from concourse.bass_utils import run_bass_kernel_spmd

N_CORES = 8
D = 512  # d_model
H = 1408  # hidden
TWOH = 2 * H
TPE = 4096  # tokens per expert
NT = 512  # token block (matmul moving free dim, one PSUM bank in f32)
KD = D // 128  # 4 contraction tiles over d
KH = H // 128  # 11 contraction tiles over h
NB = TPE // NT  # token blocks
NTILE = 2 * KH  # 22 stationary tile-groups of w12 in consumption order

F16 = mybir.dt.float16
F32 = mybir.dt.float32
NP_F16 = np.dtype(np.float16)

N_WARMUP = 48  # LDW+MM pairs on scratch data before the real stream

# Results of a traced run (test harness reads these).
last_exec_time_ns = None
last_trace_path = None


def _build():
    # Bacc (not plain Bass): its compile() pass pipeline legalizes sync
    # waits (>=2 waits per instruction are split into event-sem chains),
    # which this image's walrus requires.
    nc = bacc.Bacc("TRN2", target_bir_lowering=False, debug=False, num_devices=N_CORES)
    # x block-major [p][tb][kd][t]: 4KB contiguous per partition per block
    # (vs 1KB for the plain transpose) — 4x fewer DMA descriptors
    xp = nc.dram_tensor("xp", [128, NB * KD * NT], F16, kind="ExternalInput")
    w12p = nc.dram_tensor("w12p", [128, NTILE * KD * 128], F16, kind="ExternalInput")
    w3t = nc.dram_tensor("w3t", [H, D], F16, kind="ExternalInput")
    outT = nc.dram_tensor("outT", [D, TPE], F16, kind="ExternalOutput")

    with tile.TileContext(nc) as tc:
        with (
            tc.tile_pool(name="weights", bufs=1) as wpool,
            tc.tile_pool(name="xin", bufs=1) as xpool,
            tc.tile_pool(name="ht", bufs=2) as hpool,
            tc.tile_pool(name="swi", bufs=4) as spool,
            tc.tile_pool(name="ot", bufs=6) as opool,
            tc.tile_pool(name="pg", bufs=2, space=bass.MemorySpace.PSUM) as pgate,
            tc.tile_pool(name="pu", bufs=2, space=bass.MemorySpace.PSUM) as pup,
            tc.tile_pool(name="po", bufs=1, space=bass.MemorySpace.PSUM) as pacc,
        ):
            # w12s[p, t, kd, c]: t = 2*hh (gate) / 2*hh+1 (up)
            w12s = wpool.tile([128, NTILE, KD, 128], F16)
            w3s = wpool.tile([128, KH, D], F16)
            xs = xpool.tile([128, KD, TPE], F16)
            warm = wpool.tile([128, 128], F16)

            xp_r = xp[:, :].rearrange("p (tb kd t) -> p tb kd t", kd=KD, t=NT)
            w12_r = w12p[:, :].rearrange("p (t kd c) -> p t kd c", kd=KD, c=128)
            w3_r = w3t[:, :].rearrange("(kh p) d -> p kh d", p=128)
            outT_r = outT[:, :].rearrange("(do p) t -> p do t", p=128)

            # Warm-up scratch init on DVE (idle at start) so neither DMA
            # sequencer is delayed and the PE warm-up matmuls' dependency
            # resolves during the prologue.
            nc.vector.memset(warm[:], 0)

            # Each DMA_DIRECT2D costs ~650ns of sequencer issue time and the
            # two dynamic queues (sync=SP, gpsimd=Pool/SWDGE) split ~358GB/s
            # of HBM read bandwidth, so chunks are sized/ordered to land just
            # ahead of PE consumption: w12 tile t is consumed at roughly
            # first_mm + 0.87*t us, w3 at +19us, x block tb at +28.2*tb us.
            # x0 as ONE 512KB transfer on gpsimd, alone until it completes
            # (early aggregate DMA bw is only ~250-300GB/s while both queues
            # ramp; anything sharing the queue delays the first real matmul;
            # x0 lands at ~13.1us +/- 1us of run-to-run jitter and the
            # warm-up matmul count is sized to cover that window — a PE idle
            # >1.7us here can trip HAM's free-running MID window and
            # re-throttle the PE to 1.2GHz for 3.4us).  w12 tiles split
            # across BOTH queues in consumption order; w3 and x1..7 queue
            # behind (w3 isn't needed until block 0's GEMM2).
            def dma_w12(q, t0, t1):
                q.dma_start(out=w12s[:, t0:t1, :, :], in_=w12_r[:, t0:t1, :, :])

            def dma_x(tb):
                nc.gpsimd.dma_start(
                    out=xs[:, :, tb * NT : (tb + 1) * NT], in_=xp_r[:, tb, :, :]
                )

            dma_x(0)
            for t0, t1 in [(0, 1), (1, 2), (2, 4), (6, 10), (14, 18)]:
                dma_w12(nc.sync, t0, t1)
            for t0, t1 in [(4, 6), (10, 14), (18, NTILE)]:
                dma_w12(nc.gpsimd, t0, t1)
            nc.gpsimd.dma_start(out=w3s[:, 0:6, :], in_=w3_r[:, 0:6, :])
            nc.gpsimd.dma_start(out=w3s[:, 6:KH, :], in_=w3_r[:, 6:KH, :])
            for tb in range(1, NB):
                dma_x(tb)

            # Warm-up: PE activity with no DMA dependency, issued first so it
            # runs during the prologue/first-chunk window and flips HAM to
            # 8/8 before the real matmuls.  Garbage values, never read;
            # shares the ps_g tag so it cycles inside pgate's 2 banks.
            wps = pgate.tile([128, NT], F32, name="ps_g", tag="ps_g")
            for _ in range(N_WARMUP):
                nc.tensor.matmul(wps[:, 0:128], warm[:], warm[:], start=True, stop=True)

            for tb in range(NB):
                tsl = bass.ts(tb, NT)
                ht = hpool.tile([128, KH, NT], F16)
                acc = [
                    pacc.tile([128, NT], F32, name=f"acc{do}", tag=f"acc{do}")
                    for do in range(KD)
                ]

                # GEMM1 + SwiGLU, hh-major, gate/up interleaved per kd
                for hh in range(KH):
                    ps_g = pgate.tile([128, NT], F32)
                    ps_u = pup.tile([128, NT], F32)
                    for kd in range(KD):
                        nc.tensor.matmul(
                            ps_g[:],
                            w12s[:, 2 * hh, kd, :],
                            xs[:, kd, tsl],
                            start=(kd == 0),
                            stop=(kd == KD - 1),
                        )
                        nc.tensor.matmul(
                            ps_u[:],
                            w12s[:, 2 * hh + 1, kd, :],
                            xs[:, kd, tsl],
                            start=(kd == 0),
                            stop=(kd == KD - 1),
                        )
                    sil = spool.tile([128, NT], F32)
                    nc.scalar.activation(
                        sil[:], ps_g[:], mybir.ActivationFunctionType.Silu
                    )
                    nc.vector.tensor_mul(ht[:, hh, :], sil[:], ps_u[:])

                # GEMM2 do-major: each acc bank finishes 11 matmuls before the
                # next starts, so its copy+store overlap the remaining stream.
                # The kh chain ends at kh=10 whose ht lands ~1.4us after the
                # last GEMM1 matmul — covered by the 10 preceding matmuls.
                for do in range(KD):
                    last = tb == NB - 1 and do == KD - 1
                    ot = opool.tile([128, NT], F16)
                    hn = NT // 2
                    t0 = tb * NT
                    if not last:
                        for kh in range(KH):
                            nc.tensor.matmul(
                                acc[do][:],
                                w3s[:, kh, do * 128 : (do + 1) * 128],
                                ht[:, kh, :],
                                start=(kh == 0),
                                stop=(kh == KH - 1),
                            )
                        # PSUM copies on one engine each (bass serializes
                        # same-bank ACT+DVE access), alternating engines
                        if do % 2 == 0:
                            nc.scalar.copy(ot[:], acc[do][:])
                        else:
                            nc.vector.tensor_copy(ot[:], acc[do][:])
                        nc.sync.dma_start(out=outT_r[:, do, tsl], in_=ot[:])
                    else:
                        # the very last chain runs as two 256-token halves so
                        # the first half's copy+store overlap the second
                        # half's matmuls and only ~32KB trails the final MM.
                        # Half B accumulates in acc[0]'s bank (copied out 33
                        # matmuls ago) — PE may not write a bank a copy is
                        # still reading.  Both halves store via the sync
                        # queue — it has been streaming stores all along; the
                        # gpsimd queue is cold and costs ~1us to re-ramp.
                        for (lo, hi), pt, eng in (
                            ((0, hn), acc[do], nc.scalar),
                            ((hn, NT), acc[0], nc.vector),
                        ):
                            for kh in range(KH):
                                nc.tensor.matmul(
                                    pt[:, 0:hn],
                                    w3s[:, kh, do * 128 : (do + 1) * 128],
                                    ht[:, kh, lo:hi],
                                    start=(kh == 0),
                                    stop=(kh == KH - 1),
                                )
                            if eng is nc.scalar:
                                eng.copy(ot[:, lo:hi], pt[:, 0:hn])
                            else:
                                eng.tensor_copy(ot[:, lo:hi], pt[:, 0:hn])
                            nc.sync.dma_start(
                                out=outT_r[:, do, t0 + lo : t0 + hi],
                                in_=ot[:, lo:hi],
                            )
    nc.compile()
    return nc


_nc_cache = None


def _get_nc():
    global _nc_cache
    if _nc_cache is None:
        _nc_cache = _build()
    return _nc_cache


def _pack_x(xe):
    """[TPE, D] -> [128, NB*KD*NT] block-major: xp[p, (tb, kd, t)] =
    xe.T[kd*128 + p, tb*NT + t], 4KB contiguous per partition per block."""
    a = np.ascontiguousarray(xe.T).astype(NP_F16)  # [D, TPE]
    a = a.reshape(KD, 128, NB, NT).transpose(1, 2, 0, 3)  # [p, tb, kd, t]
    return np.ascontiguousarray(a.reshape(128, NB * KD * NT))


def _pack_w12(w12_e):
    """[2816, 512] -> [128, 22*4*128] in PE consumption order.

    w12p[p, (t, kd, c)] = w12_e.T[kd*128 + p, col(t, c)] with
    col(2*hh, c) = hh*128 + c (gate), col(2*hh+1, c) = H + hh*128 + c (up).
    """
    a = np.ascontiguousarray(w12_e.T).astype(NP_F16)  # [D, 2H]
    a = a.reshape(KD, 128, 2, KH, 128)  # [kd, p, gu, hh, c]
    a = a.transpose(1, 3, 2, 0, 4)  # [p, hh, gu, kd, c]
    return np.ascontiguousarray(a.reshape(128, NTILE * KD * 128))


def kernel(sorted_x, w12, w3, expert_starts, expert_ends):
    global last_exec_time_ns, last_trace_path
    sorted_x = np.asarray(sorted_x)
    w12 = np.asarray(w12)
    w3 = np.asarray(w3)
    starts = np.asarray(expert_starts).astype(np.int64)
    T = sorted_x.shape[0]

    in_maps = []
    for e in range(N_CORES):
        # jax.lax.dynamic_slice clamps the start index the same way
        s = int(min(max(starts[e], 0), T - TPE))
        xe = sorted_x[s : s + TPE]  # (TPE, D) f32
        in_maps.append(
            {
                "xp": _pack_x(xe),
                "w12p": _pack_w12(w12[e]),
                "w3t": np.ascontiguousarray(w3[e].T).astype(NP_F16),
            }
        )

    trace = bool(os.environ.get("BASS_MOE_TRACE"))
    res = run_bass_kernel_spmd(
        _get_nc(), in_maps, core_ids=list(range(N_CORES)), trace=trace
    )
    if trace:
        last_exec_time_ns = res.exec_time_ns
        iat = res.instructions_and_trace
        last_trace_path = iat[1] if iat else None

    out = np.empty((N_CORES * TPE, D), dtype=np.float32)
    for e in range(N_CORES):
        out[e * TPE : (e + 1) * TPE] = res.results[e]["outT"].T.astype(np.float32)
    return out


# revision 32
# speedup vs baseline: 1.0092x; 1.0001x over previous
"""Grouped SwiGLU expert MLP (MoE) on 8 Trainium2 NeuronCores.

Problem: sorted_x [32768, 512] f32, tokens pre-sorted by expert into 8 equal
contiguous segments of 4096 tokens; per-expert SwiGLU MLP
    h12 = x_e @ w12[e].T          (4096, 2816)
    h   = silu(h12[:, :1408]) * h12[:, 1408:]
    out = h @ w3[e].T             (4096, 512)

Sharding: pure expert parallelism — core e owns expert e's weights and its
4096-token segment (sliced host-side from expert_starts), so no device-side
collectives are needed; the host concatenates the per-core outputs.

Device layout is feature-major throughout ("contraction dim on partitions"),
which makes both GEMMs transpose-free on chip:
    xt   = x_e.T   [512, 4096]  fp16
    w12p = w12.T   pre-tiled    fp16  (see below)
    w3t  = w3.T    [1408, 512]  fp16
    outT = out.T   [512, 4096]  fp16  (host transposes + upcasts back)
GEMM1 produces H12^T tiles [128h, Nt] (PSUM f32), SwiGLU runs on ACT+DVE into
fp16 H^T tiles, GEMM2 consumes them directly. fp16 operands run the PE at
1 cycle/row; accumulation is always f32 in PSUM.

The PE stream (1056 matmuls x 512 moving rows = 225.3us warm) is the hard
floor; everything else is edge management:
  * warm-up matmuls on a memset scratch tile run during the fixed ~6.6us
    NEFF prologue + first-DMA window, so HAM un-throttles (1.2->2.4 GHz)
    before the real stream begins and the PE never sits idle at the head;
  * w12 is repacked host-side into PE consumption order — 22 stationary
    tile-groups t=(g0,u0,g1,u1,...) of [kd][128 cols], 128KB each with 1KB
    contiguous runs per partition (column-sliced chunks of the plain
    transposed layout have 256B runs and run at ~4x lower DMA bandwidth,
    which starved the PE for ~10us at the head in earlier revisions);
  * input DMAs are split/ordered by consumption time across the two queues
    (sync=SP: w12 tile stream + output stores; gpsimd=SWDGE: x0 per-kd,
    then w3, then x1..7);
  * GEMM2 runs do-major after each block's hh loop (not LAG-interleaved),
    so acc[do] banks finish one at a time and the PSUM->SBUF copy + output
    DMA of each do overlaps the remaining matmul stream; copies split in
    half across ACT and DVE; the very last store is split across both DMA
    queues so only ~64KB trails the final matmul;
  * outputs store as fp16 (adds ~1e-4 rel err vs the 2e-2 budget).
"""

import os

import numpy as np
import ml_dtypes

import concourse.bass as bass
import concourse.mybir as mybir
import concourse.tile as tile
from concourse import bacc
from concourse.bass_utils import run_bass_kernel_spmd

N_CORES = 8
D = 512  # d_model
H = 1408  # hidden
TWOH = 2 * H
TPE = 4096  # tokens per expert
NT = 512  # token block (matmul moving free dim, one PSUM bank in f32)
KD = D // 128  # 4 contraction tiles over d
KH = H // 128  # 11 contraction tiles over h
NB = TPE // NT  # token blocks
NTILE = 2 * KH  # 22 stationary tile-groups of w12 in consumption order

F16 = mybir.dt.float16
F32 = mybir.dt.float32
NP_F16 = np.dtype(np.float16)

N_WARMUP = 60  # LDW+MM pairs on scratch data before the real stream

# Results of a traced run (test harness reads these).
last_exec_time_ns = None
last_trace_path = None


def _build():
    # Bacc (not plain Bass): its compile() pass pipeline legalizes sync
    # waits (>=2 waits per instruction are split into event-sem chains),
    # which this image's walrus requires.
    nc = bacc.Bacc("TRN2", target_bir_lowering=False, debug=False, num_devices=N_CORES)
    # x block-major [p][tb][kd][t]: 4KB contiguous per partition per block
    # (vs 1KB for the plain transpose) — 4x fewer DMA descriptors
    xp = nc.dram_tensor("xp", [128, NB * KD * NT], F16, kind="ExternalInput")
    w12p = nc.dram_tensor("w12p", [128, NTILE * KD * 128], F16, kind="ExternalInput")
    w3t = nc.dram_tensor("w3t", [H, D], F16, kind="ExternalInput")
    outT = nc.dram_tensor("outT", [D, TPE], F16, kind="ExternalOutput")

    with tile.TileContext(nc) as tc:
        with (
            tc.tile_pool(name="weights", bufs=1) as wpool,
            tc.tile_pool(name="xin", bufs=1) as xpool,
            tc.tile_pool(name="ht", bufs=2) as hpool,
            tc.tile_pool(name="swi", bufs=4) as spool,
            tc.tile_pool(name="ot", bufs=6) as opool,
            tc.tile_pool(name="pg", bufs=2, space=bass.MemorySpace.PSUM) as pgate,
            tc.tile_pool(name="pu", bufs=2, space=bass.MemorySpace.PSUM) as pup,
            tc.tile_pool(name="po", bufs=1, space=bass.MemorySpace.PSUM) as pacc,
        ):
            # w12s[p, t, kd, c]: t = 2*hh (gate) / 2*hh+1 (up)
            w12s = wpool.tile([128, NTILE, KD, 128], F16)
            w3s = wpool.tile([128, KH, D], F16)
            xs = xpool.tile([128, KD, TPE], F16)
            warm = wpool.tile([128, 128], F16)

            xp_r = xp[:, :].rearrange("p (tb kd t) -> p tb kd t", kd=KD, t=NT)
            w12_r = w12p[:, :].rearrange("p (t kd c) -> p t kd c", kd=KD, c=128)
            w3_r = w3t[:, :].rearrange("(kh p) d -> p kh d", p=128)
            outT_r = outT[:, :].rearrange("(do p) t -> p do t", p=128)

            # Warm-up scratch init on DVE (idle at start) so neither DMA
            # sequencer is delayed and the PE warm-up matmuls' dependency
            # resolves during the prologue.
            nc.vector.memset(warm[:], 0)

            # Each DMA_DIRECT2D costs ~650ns of sequencer issue time and the
            # two dynamic queues (sync=SP, gpsimd=Pool/SWDGE) split ~358GB/s
            # of HBM read bandwidth, so chunks are sized/ordered to land just
            # ahead of PE consumption: w12 tile t is consumed at roughly
            # first_mm + 0.87*t us, w3 at +19us, x block tb at +28.2*tb us.
            # x0 as ONE 512KB transfer on gpsimd, alone until it completes
            # (early aggregate DMA bw is only ~250-300GB/s while both queues
            # ramp; anything sharing the queue delays the first real matmul;
            # x0 lands at ~13.1us +/- 1us of run-to-run jitter and the
            # warm-up matmul count is sized to cover that window — a PE idle
            # >1.7us here can trip HAM's free-running MID window and
            # re-throttle the PE to 1.2GHz for 3.4us).  w12 tiles split
            # across BOTH queues in consumption order; w3 and x1..7 queue
            # behind (w3 isn't needed until block 0's GEMM2).
            def dma_w12(q, t0, t1):
                q.dma_start(out=w12s[:, t0:t1, :, :], in_=w12_r[:, t0:t1, :, :])

            def dma_x(tb):
                nc.gpsimd.dma_start(
                    out=xs[:, :, tb * NT : (tb + 1) * NT], in_=xp_r[:, tb, :, :]
                )

            dma_x(0)
            for t0, t1 in [(0, 1), (1, 2), (2, 4), (6, 10), (14, 18)]:
                dma_w12(nc.sync, t0, t1)
            for t0, t1 in [(4, 6), (10, 14), (18, NTILE)]:
                dma_w12(nc.gpsimd, t0, t1)
            nc.gpsimd.dma_start(out=w3s[:, 0:6, :], in_=w3_r[:, 0:6, :])
            nc.gpsimd.dma_start(out=w3s[:, 6:KH, :], in_=w3_r[:, 6:KH, :])
            for tb in range(1, NB):
                dma_x(tb)

            # Warm-up: PE activity with no DMA dependency, issued first so it
            # runs during the prologue/first-chunk window and flips HAM to
            # 8/8 before the real matmuls.  Garbage values, never read;
            # shares the ps_g tag so it cycles inside pgate's 2 banks.
            wps = pgate.tile([128, NT], F32, name="ps_g", tag="ps_g")
            for _ in range(N_WARMUP):
                nc.tensor.matmul(wps[:, 0:128], warm[:], warm[:], start=True, stop=True)

            for tb in range(NB):
                tsl = bass.ts(tb, NT)
                ht = hpool.tile([128, KH, NT], F16)
                acc = [
                    pacc.tile([128, NT], F32, name=f"acc{do}", tag=f"acc{do}")
                    for do in range(KD)
                ]

                # GEMM1 + SwiGLU, hh-major, gate/up interleaved per kd
                for hh in range(KH):
                    ps_g = pgate.tile([128, NT], F32)
                    ps_u = pup.tile([128, NT], F32)
                    for kd in range(KD):
                        nc.tensor.matmul(
                            ps_g[:],
                            w12s[:, 2 * hh, kd, :],
                            xs[:, kd, tsl],
                            start=(kd == 0),
                            stop=(kd == KD - 1),
                        )
                        nc.tensor.matmul(
                            ps_u[:],
                            w12s[:, 2 * hh + 1, kd, :],
                            xs[:, kd, tsl],
                            start=(kd == 0),
                            stop=(kd == KD - 1),
                        )
                    sil = spool.tile([128, NT], F32)
                    nc.scalar.activation(
                        sil[:], ps_g[:], mybir.ActivationFunctionType.Silu
                    )
                    nc.vector.tensor_mul(ht[:, hh, :], sil[:], ps_u[:])

                # GEMM2 do-major: each acc bank finishes 11 matmuls before the
                # next starts, so its copy+store overlap the remaining stream.
                # The kh chain ends at kh=10 whose ht lands ~1.4us after the
                # last GEMM1 matmul — covered by the 10 preceding matmuls.
                for do in range(KD):
                    last = tb == NB - 1 and do == KD - 1
                    ot = opool.tile([128, NT], F16)
                    hn = NT // 2
                    t0 = tb * NT
                    if not last:
                        for kh in range(KH):
                            nc.tensor.matmul(
                                acc[do][:],
                                w3s[:, kh, do * 128 : (do + 1) * 128],
                                ht[:, kh, :],
                                start=(kh == 0),
                                stop=(kh == KH - 1),
                            )
                        # PSUM copies on one engine each (bass serializes
                        # same-bank ACT+DVE access), alternating engines
                        if do % 2 == 0:
                            nc.scalar.copy(ot[:], acc[do][:])
                        else:
                            nc.vector.tensor_copy(ot[:], acc[do][:])
                        nc.sync.dma_start(out=outT_r[:, do, tsl], in_=ot[:])
                    else:
                        # the very last chain runs as two 256-token halves so
                        # the first half's copy+store overlap the second
                        # half's matmuls and only ~32KB trails the final MM.
                        # Half B accumulates in acc[0]'s bank (copied out 33
                        # matmuls ago) — PE may not write a bank a copy is
                        # still reading.  Both halves store via the sync
                        # queue — it has been streaming stores all along; the
                        # gpsimd queue is cold and costs ~1us to re-ramp.
                        for (lo, hi), pt, eng in (
                            ((0, hn), acc[do], nc.scalar),
                            ((hn, NT), acc[0], nc.vector),
                        ):
                            for kh in range(KH):
                                nc.tensor.matmul(
                                    pt[:, 0:hn],
                                    w3s[:, kh, do * 128 : (do + 1) * 128],
                                    ht[:, kh, lo:hi],
                                    start=(kh == 0),
                                    stop=(kh == KH - 1),
                                )
                            if eng is nc.scalar:
                                eng.copy(ot[:, lo:hi], pt[:, 0:hn])
                            else:
                                eng.tensor_copy(ot[:, lo:hi], pt[:, 0:hn])
                            nc.sync.dma_start(
                                out=outT_r[:, do, t0 + lo : t0 + hi],
                                in_=ot[:, lo:hi],
                            )
    nc.compile()
    return nc


_nc_cache = None


def _get_nc():
    global _nc_cache
    if _nc_cache is None:
        _nc_cache = _build()
    return _nc_cache


def _pack_x(xe):
    """[TPE, D] -> [128, NB*KD*NT] block-major: xp[p, (tb, kd, t)] =
    xe.T[kd*128 + p, tb*NT + t], 4KB contiguous per partition per block."""
    a = np.ascontiguousarray(xe.T).astype(NP_F16)  # [D, TPE]
    a = a.reshape(KD, 128, NB, NT).transpose(1, 2, 0, 3)  # [p, tb, kd, t]
    return np.ascontiguousarray(a.reshape(128, NB * KD * NT))


def _pack_w12(w12_e):
    """[2816, 512] -> [128, 22*4*128] in PE consumption order.

    w12p[p, (t, kd, c)] = w12_e.T[kd*128 + p, col(t, c)] with
    col(2*hh, c) = hh*128 + c (gate), col(2*hh+1, c) = H + hh*128 + c (up).
    """
    a = np.ascontiguousarray(w12_e.T).astype(NP_F16)  # [D, 2H]
    a = a.reshape(KD, 128, 2, KH, 128)  # [kd, p, gu, hh, c]
    a = a.transpose(1, 3, 2, 0, 4)  # [p, hh, gu, kd, c]
    return np.ascontiguousarray(a.reshape(128, NTILE * KD * 128))


def kernel(sorted_x, w12, w3, expert_starts, expert_ends):
    global last_exec_time_ns, last_trace_path
    sorted_x = np.asarray(sorted_x)
    w12 = np.asarray(w12)
    w3 = np.asarray(w3)
    starts = np.asarray(expert_starts).astype(np.int64)
    T = sorted_x.shape[0]

    in_maps = []
    for e in range(N_CORES):
        # jax.lax.dynamic_slice clamps the start index the same way
        s = int(min(max(starts[e], 0), T - TPE))
        xe = sorted_x[s : s + TPE]  # (TPE, D) f32
        in_maps.append(
            {
                "xp": _pack_x(xe),
                "w12p": _pack_w12(w12[e]),
                "w3t": np.ascontiguousarray(w3[e].T).astype(NP_F16),
            }
        )

    trace = bool(os.environ.get("BASS_MOE_TRACE"))
    res = run_bass_kernel_spmd(
        _get_nc(), in_maps, core_ids=list(range(N_CORES)), trace=trace
    )
    if trace:
        last_exec_time_ns = res.exec_time_ns
        iat = res.instructions_and_trace
        last_trace_path = iat[1] if iat else None

    out = np.empty((N_CORES * TPE, D), dtype=np.float32)
    for e in range(N_CORES):
        out[e * TPE : (e + 1) * TPE] = res.results[e]["outT"].T.astype(np.float32)
    return out


# revision 33
# speedup vs baseline: 1.0146x; 1.0053x over previous
"""Grouped SwiGLU expert MLP (MoE) on 8 Trainium2 NeuronCores.

Problem: sorted_x [32768, 512] f32, tokens pre-sorted by expert into 8 equal
contiguous segments of 4096 tokens; per-expert SwiGLU MLP
    h12 = x_e @ w12[e].T          (4096, 2816)
    h   = silu(h12[:, :1408]) * h12[:, 1408:]
    out = h @ w3[e].T             (4096, 512)

Sharding: pure expert parallelism — core e owns expert e's weights and its
4096-token segment (sliced host-side from expert_starts), so no device-side
collectives are needed; the host concatenates the per-core outputs.

Device layout is feature-major throughout ("contraction dim on partitions"),
which makes both GEMMs transpose-free on chip:
    xp   = x_e.T   block-major pre-tiled fp16 (4KB lines, see _pack_x)
    w12p = w12.T   pre-tiled    fp16  (see _pack_w12)
    w3t  = w3.T    [1408, 512]  fp16
    outT = out.T   [512, 4096]  fp16  (host transposes + upcasts back)
GEMM1 produces H12^T tiles [128h, Nt] (PSUM f32), SwiGLU runs on ACT+DVE into
fp16 H^T tiles, GEMM2 consumes them directly. fp16 operands run the PE at
1 cycle/row; accumulation is always f32 in PSUM.

The PE stream (1056 matmuls x 512 moving rows = 225.3us warm) is the hard
floor; everything else is edge management:
  * warm-up matmuls on a memset scratch tile run during the fixed ~6.6us
    NEFF prologue + first-DMA window, so HAM un-throttles (1.2->2.4 GHz)
    before the real stream begins and the PE never sits idle at the head;
  * w12 is repacked host-side into PE consumption order — 22 stationary
    tile-groups t=(g0,u0,g1,u1,...) of [kd][128 cols], 128KB each with 1KB
    contiguous runs per partition (column-sliced chunks of the plain
    transposed layout have 256B runs and run at ~4x lower DMA bandwidth,
    which starved the PE for ~10us at the head in earlier revisions);
  * input DMAs are split/ordered by consumption time across the two queues
    (sync=SP: w12 tile stream + output stores; gpsimd=SWDGE: x0 per-kd,
    then w3, then x1..7);
  * GEMM2 runs do-major after each block's hh loop (not LAG-interleaved),
    so acc[do] banks finish one at a time and the PSUM->SBUF copy + output
    DMA of each do overlaps the remaining matmul stream; copies split in
    half across ACT and DVE; the very last store is split across both DMA
    queues so only ~64KB trails the final matmul;
  * outputs store as fp16 (adds ~1e-4 rel err vs the 2e-2 budget).
"""

import os

import numpy as np
import ml_dtypes

import concourse.bass as bass
import concourse.mybir as mybir
import concourse.tile as tile
from concourse import bacc
from concourse.bass_utils import run_bass_kernel_spmd

N_CORES = 8
D = 512  # d_model
H = 1408  # hidden
TWOH = 2 * H
TPE = 4096  # tokens per expert
NT = 512  # token block (matmul moving free dim, one PSUM bank in f32)
KD = D // 128  # 4 contraction tiles over d
KH = H // 128  # 11 contraction tiles over h
NB = TPE // NT  # token blocks
NTILE = 2 * KH  # 22 stationary tile-groups of w12 in consumption order

F16 = mybir.dt.float16
F32 = mybir.dt.float32
NP_F16 = np.dtype(np.float16)

N_WARMUP = 60  # LDW+MM pairs on scratch data before the real stream

# Results of a traced run (test harness reads these).
last_exec_time_ns = None
last_trace_path = None


def _build():
    # Bacc (not plain Bass): its compile() pass pipeline legalizes sync
    # waits (>=2 waits per instruction are split into event-sem chains),
    # which this image's walrus requires.
    nc = bacc.Bacc("TRN2", target_bir_lowering=False, debug=False, num_devices=N_CORES)
    # x block-major [p][tb][kd][t]: 4KB contiguous per partition per block
    # (vs 1KB for the plain transpose) — 4x fewer DMA descriptors
    xp = nc.dram_tensor("xp", [128, NB * KD * NT], F16, kind="ExternalInput")
    w12p = nc.dram_tensor("w12p", [128, NTILE * KD * 128], F16, kind="ExternalInput")
    w3t = nc.dram_tensor("w3t", [H, D], F16, kind="ExternalInput")
    outT = nc.dram_tensor("outT", [D, TPE], F16, kind="ExternalOutput")

    with tile.TileContext(nc) as tc:
        with (
            tc.tile_pool(name="weights", bufs=1) as wpool,
            tc.tile_pool(name="xin", bufs=1) as xpool,
            tc.tile_pool(name="ht", bufs=2) as hpool,
            tc.tile_pool(name="swi", bufs=4) as spool,
            tc.tile_pool(name="ot", bufs=6) as opool,
            tc.tile_pool(name="pg", bufs=2, space=bass.MemorySpace.PSUM) as pgate,
            tc.tile_pool(name="pu", bufs=2, space=bass.MemorySpace.PSUM) as pup,
            tc.tile_pool(name="po", bufs=1, space=bass.MemorySpace.PSUM) as pacc,
        ):
            # w12s[p, t, kd, c]: t = 2*hh (gate) / 2*hh+1 (up)
            w12s = wpool.tile([128, NTILE, KD, 128], F16)
            w3s = wpool.tile([128, KH, D], F16)
            xs = xpool.tile([128, KD, TPE], F16)
            warm = wpool.tile([128, 128], F16)

            xp_r = xp[:, :].rearrange("p (tb kd t) -> p tb kd t", kd=KD, t=NT)
            w12_r = w12p[:, :].rearrange("p (t kd c) -> p t kd c", kd=KD, c=128)
            w3_r = w3t[:, :].rearrange("(kh p) d -> p kh d", p=128)
            outT_r = outT[:, :].rearrange("(do p) t -> p do t", p=128)

            # Warm-up scratch init on DVE (idle at start) so neither DMA
            # sequencer is delayed and the PE warm-up matmuls' dependency
            # resolves during the prologue.
            nc.vector.memset(warm[:], 0)

            # Each DMA_DIRECT2D costs ~650ns of sequencer issue time and the
            # two dynamic queues (sync=SP, gpsimd=Pool/SWDGE) split ~358GB/s
            # of HBM read bandwidth, so chunks are sized/ordered to land just
            # ahead of PE consumption: w12 tile t is consumed at roughly
            # first_mm + 0.87*t us, w3 at +19us, x block tb at +28.2*tb us.
            # x0 as ONE 512KB transfer on gpsimd, alone until it completes
            # (early aggregate DMA bw is only ~250-300GB/s while both queues
            # ramp; anything sharing the queue delays the first real matmul;
            # x0 lands at ~13.1us +/- 1us of run-to-run jitter and the
            # warm-up matmul count is sized to cover that window — a PE idle
            # >1.7us here can trip HAM's free-running MID window and
            # re-throttle the PE to 1.2GHz for 3.4us).  w12 tiles split
            # across BOTH queues in consumption order; w3 and x1..7 queue
            # behind (w3 isn't needed until block 0's GEMM2).
            def dma_w12(q, t0, t1):
                q.dma_start(out=w12s[:, t0:t1, :, :], in_=w12_r[:, t0:t1, :, :])

            def dma_x(tb):
                nc.gpsimd.dma_start(
                    out=xs[:, :, tb * NT : (tb + 1) * NT], in_=xp_r[:, tb, :, :]
                )

            dma_x(0)
            for t0, t1 in [(0, 1), (1, 2), (2, 4), (6, 10), (14, 18)]:
                dma_w12(nc.sync, t0, t1)
            for t0, t1 in [(4, 6), (10, 14), (18, NTILE)]:
                dma_w12(nc.gpsimd, t0, t1)
            nc.gpsimd.dma_start(out=w3s[:, 0:6, :], in_=w3_r[:, 0:6, :])
            nc.gpsimd.dma_start(out=w3s[:, 6:KH, :], in_=w3_r[:, 6:KH, :])
            for tb in range(1, NB):
                dma_x(tb)

            # Warm-up: PE activity with no DMA dependency, issued first so it
            # runs during the prologue/first-chunk window and flips HAM to
            # 8/8 before the real matmuls.  Garbage values, never read;
            # shares the ps_g tag so it cycles inside pgate's 2 banks.
            wps = pgate.tile([128, NT], F32, name="ps_g", tag="ps_g")
            for _ in range(N_WARMUP):
                nc.tensor.matmul(wps[:, 0:128], warm[:], warm[:], start=True, stop=True)

            for tb in range(NB):
                tsl = bass.ts(tb, NT)
                ht = hpool.tile([128, KH, NT], F16)
                acc = [
                    pacc.tile([128, NT], F32, name=f"acc{do}", tag=f"acc{do}")
                    for do in range(KD)
                ]

                # GEMM1 + SwiGLU, hh-major, gate/up interleaved per kd
                for hh in range(KH):
                    ps_g = pgate.tile([128, NT], F32)
                    ps_u = pup.tile([128, NT], F32)
                    for kd in range(KD):
                        nc.tensor.matmul(
                            ps_g[:],
                            w12s[:, 2 * hh, kd, :],
                            xs[:, kd, tsl],
                            start=(kd == 0),
                            stop=(kd == KD - 1),
                        )
                        nc.tensor.matmul(
                            ps_u[:],
                            w12s[:, 2 * hh + 1, kd, :],
                            xs[:, kd, tsl],
                            start=(kd == 0),
                            stop=(kd == KD - 1),
                        )
                    sil = spool.tile([128, NT], F32)
                    nc.scalar.activation(
                        sil[:], ps_g[:], mybir.ActivationFunctionType.Silu
                    )
                    nc.vector.tensor_mul(ht[:, hh, :], sil[:], ps_u[:])

                # GEMM2 do-major: each acc bank finishes 11 matmuls before the
                # next starts, so its copy+store overlap the remaining stream.
                # The kh chain ends at kh=10 whose ht lands ~1.4us after the
                # last GEMM1 matmul — covered by the 10 preceding matmuls.
                for do in range(KD):
                    last = tb == NB - 1 and do == KD - 1
                    ot = opool.tile([128, NT], F16)
                    hn = NT // 2
                    t0 = tb * NT
                    if not last:
                        for kh in range(KH):
                            nc.tensor.matmul(
                                acc[do][:],
                                w3s[:, kh, do * 128 : (do + 1) * 128],
                                ht[:, kh, :],
                                start=(kh == 0),
                                stop=(kh == KH - 1),
                            )
                        # PSUM copies on one engine each (bass serializes
                        # same-bank ACT+DVE access), alternating engines
                        if do % 2 == 0:
                            nc.scalar.copy(ot[:], acc[do][:])
                        else:
                            nc.vector.tensor_copy(ot[:], acc[do][:])
                        nc.sync.dma_start(out=outT_r[:, do, tsl], in_=ot[:])
                    else:
                        # the very last chain runs as two 256-token halves so
                        # the first half's copy+store overlap the second
                        # half's matmuls and only ~32KB trails the final MM.
                        # Half B accumulates in acc[0]'s bank (copied out 33
                        # matmuls ago) — PE may not write a bank a copy is
                        # still reading.  Both halves store via the sync
                        # queue — it has been streaming stores all along; the
                        # gpsimd queue is cold and costs ~1us to re-ramp.
                        for (lo, hi), pt, eng in (
                            ((0, hn), acc[do], nc.scalar),
                            ((hn, NT), acc[0], nc.vector),
                        ):
                            for kh in range(KH):
                                nc.tensor.matmul(
                                    pt[:, 0:hn],
                                    w3s[:, kh, do * 128 : (do + 1) * 128],
                                    ht[:, kh, lo:hi],
                                    start=(kh == 0),
                                    stop=(kh == KH - 1),
                                )
                            if eng is nc.scalar:
                                eng.copy(ot[:, lo:hi], pt[:, 0:hn])
                            else:
                                eng.tensor_copy(ot[:, lo:hi], pt[:, 0:hn])
                            nc.sync.dma_start(
                                out=outT_r[:, do, t0 + lo : t0 + hi],
                                in_=ot[:, lo:hi],
                            )
    nc.compile()
    return nc


_nc_cache = None


def _get_nc():
    global _nc_cache
    if _nc_cache is None:
        _nc_cache = _build()
    return _nc_cache


def _pack_x(xe):
    """[TPE, D] -> [128, NB*KD*NT] block-major: xp[p, (tb, kd, t)] =
    xe.T[kd*128 + p, tb*NT + t], 4KB contiguous per partition per block."""
    a = np.ascontiguousarray(xe.T).astype(NP_F16)  # [D, TPE]
    a = a.reshape(KD, 128, NB, NT).transpose(1, 2, 0, 3)  # [p, tb, kd, t]
    return np.ascontiguousarray(a.reshape(128, NB * KD * NT))


def _pack_w12(w12_e):
    """[2816, 512] -> [128, 22*4*128] in PE consumption order.

    w12p[p, (t, kd, c)] = w12_e.T[kd*128 + p, col(t, c)] with
    col(2*hh, c) = hh*128 + c (gate), col(2*hh+1, c) = H + hh*128 + c (up).
    """
    a = np.ascontiguousarray(w12_e.T).astype(NP_F16)  # [D, 2H]
    a = a.reshape(KD, 128, 2, KH, 128)  # [kd, p, gu, hh, c]
    a = a.transpose(1, 3, 2, 0, 4)  # [p, hh, gu, kd, c]
    return np.ascontiguousarray(a.reshape(128, NTILE * KD * 128))


def kernel(sorted_x, w12, w3, expert_starts, expert_ends):
    global last_exec_time_ns, last_trace_path
    sorted_x = np.asarray(sorted_x)
    w12 = np.asarray(w12)
    w3 = np.asarray(w3)
    starts = np.asarray(expert_starts).astype(np.int64)
    T = sorted_x.shape[0]

    in_maps = []
    for e in range(N_CORES):
        # jax.lax.dynamic_slice clamps the start index the same way
        s = int(min(max(starts[e], 0), T - TPE))
        xe = sorted_x[s : s + TPE]  # (TPE, D) f32
        in_maps.append(
            {
                "xp": _pack_x(xe),
                "w12p": _pack_w12(w12[e]),
                "w3t": np.ascontiguousarray(w3[e].T).astype(NP_F16),
            }
        )

    trace = bool(os.environ.get("BASS_MOE_TRACE"))
    res = run_bass_kernel_spmd(
        _get_nc(), in_maps, core_ids=list(range(N_CORES)), trace=trace
    )
    if trace:
        last_exec_time_ns = res.exec_time_ns
        iat = res.instructions_and_trace
        last_trace_path = iat[1] if iat else None

    out = np.empty((N_CORES * TPE, D), dtype=np.float32)
    for e in range(N_CORES):
        out[e * TPE : (e + 1) * TPE] = res.results[e]["outT"].T.astype(np.float32)
    return out
